# revision 29
# baseline (speedup 1.0000x reference)
"""Trainium2 Bass kernel for nn_Attention (dense transformer attention block).

Full causal attention: QKV projection + RoPE + softmax(QK^T/sqrt(d) + mask)V + WO,
bsz=1, seqlen=2048, dim=4096, 32 heads x head_dim 128, fp32 I/O.

Sharding: tensor-parallel across heads on 8 NeuronCores. Core c owns heads
4c..4c+3 (wq/wk/wv output columns, attention) and wo output columns
512c..512c+512 (after an AllGather of the per-core attn^T shard along the
head axis). Host concatenates the 8 output column shards.

All matmuls run as float32r (fp32 storage, reduced-precision single-pass PE
mode, ~1e-4 relative accuracy at full PE rate).
"""

import ml_dtypes
import numpy as np

import concourse.bacc as bacc
import concourse.mybir as mybir
import concourse.tile as tile
from concourse.bass_utils import run_bass_kernel_spmd

# Problem constants (hardcoded per contract)
N_CORES = 8
S = 2048              # sequence length
D = 4096              # model dim
HD = 128              # head dim
NH_LOC = 4            # heads per core
DSH = 512             # per-core shard width (NH_LOC * HD)
KT = D // 128         # 32 contraction tiles over model dim
QTILES = S // 128     # 16 token tiles
QRANGES = S // 512    # 4 query ranges of 512
SCALE = float(1.0 / np.sqrt(HD))

F32 = mybir.dt.float32
F32R = mybir.dt.float32r

_PROGRAMS = {}


def _build_program(mode, debug_stop=None):
    """mode: 'causal' (triu -1e9 mask), 'nomask' (zero mask), 'general'
    (arbitrary additive mask streamed from DRAM)."""
    causal = mode == "causal"
    general = mode == "general"
    do_attn = debug_stop not in ("qkv",)
    do_ag = do_attn and debug_stop not in ("attn",)
    do_wo = do_ag and debug_stop not in ("ag",)

    nc = bacc.Bacc("TRN2", target_bir_lowering=False, debug=False,
                   num_devices=N_CORES)

    BF16 = mybir.dt.bfloat16
    # ---- external inputs (per core) ----
    xT_d = nc.dram_tensor("xT", [D, S], BF16, kind="ExternalInput")
    wq_d = nc.dram_tensor("wq", [NH_LOC, 128, KT, HD], BF16, kind="ExternalInput")
    wk_d = nc.dram_tensor("wk", [NH_LOC, 128, KT, HD], BF16, kind="ExternalInput")
    wv_d = nc.dram_tensor("wv", [128, KT, DSH], BF16, kind="ExternalInput")
    wo_d = nc.dram_tensor("wo", [128, KT, DSH], BF16, kind="ExternalInput")
    fr_d = nc.dram_tensor("fr128", [128, S], BF16, kind="ExternalInput")
    fis_d = nc.dram_tensor("fis128", [128, S], BF16, kind="ExternalInput")
    perm_d = nc.dram_tensor("perm", [128, 128], F32R, kind="ExternalInput")
    onesmat_d = nc.dram_tensor("onesmat", [128, 128], F32R, kind="ExternalInput")
    if causal:
        maskt_d = nc.dram_tensor("maskt", [128, 128], F32, kind="ExternalInput")
    if general:
        masktf_d = nc.dram_tensor("masktf", [S, S], F32, kind="ExternalInput")
    out_d = nc.dram_tensor("out", [S, DSH], F32, kind="ExternalOutput")
    dbg_d = None
    if debug_stop == "qkv":
        dbg_d = nc.dram_tensor("dbg", [2 * DSH, S], BF16, kind="ExternalOutput")
    if debug_stop in ("attn", "ag"):
        dbg_d = nc.dram_tensor("dbg", [DSH, S], BF16, kind="ExternalOutput")

    with tile.TileContext(nc) as tc:
        with (
            tc.tile_pool(name="consts", bufs=1) as cns,
            tc.tile_pool(name="dram", bufs=1, space="DRAM") as dram,
            tc.tile_pool(name="akv", bufs=1) as akv,
            tc.tile_pool(name="ps", bufs=1, space="PSUM") as ps,
        ):
            qt_spill = dram.tile([DSH, S], BF16)    # Q^T rotated, [d, s]
            agi = [dram.tile([DSH, 512], BF16, name=f"agi{r}") for r in range(4)]
            ago = [dram.tile([D, 512], BF16, addr_space="Shared", name=f"ago{r}")
                   for r in range(4)]

            onesmat_sb = cns.tile([128, 128], F32R, tag="om")
            if causal:
                maskt_sb = cns.tile([128, 128], F32, tag="maskt")

            kts = [akv.tile([128, S], BF16, tag=f"kth{h}", name=f"kth{h}")
                   for h in range(NH_LOC)]
            vhs = [akv.tile([128, QTILES, 128], BF16, tag=f"vh{h}",
                            name=f"vh{h}")
                   for h in range(NH_LOC)]

            # ---------- Section 1: QK projections (x^T fully resident) ----------
            anchor = [None]
            xtqp_h = [None]

            def emit_v_quarter(vq, wvpool, xt_src=None):
                if xt_src is None:
                    xtq = xtqp_h[0].tile([128, KT, 512], BF16, tag="xtq",
                                         name=f"xtq{vq}")
                    for ch in range(2):
                        nc.sync.dma_start(
                            xtq[:, ch * 16:(ch + 1) * 16, :],
                            xT_d[ch * 2048:(ch + 1) * 2048,
                                 vq * 512:(vq + 1) * 512]
                            .rearrange("(kt p) s -> p kt s", p=128),
                        )

                    def xt_src(kt, tt):
                        return xtq[:, kt, tt * 128:(tt + 1) * 128]
                psv = [ps.tile([128, 512], F32, tag=f"a{t}",
                               name=f"vps{vq}_{t}", bufs=1)
                       for t in range(4)]
                for ktc in range(8):
                    wv_c = wvpool.tile([128, 4, 512], BF16, tag="wv")
                    nc.scalar.dma_start(
                        wv_c[:],
                        wv_d[:, ktc * 4:(ktc + 1) * 4, :],
                    )
                    for kt4 in range(4):
                        kt = ktc * 4 + kt4
                        for tt in range(4):
                            nc.tensor.matmul(
                                psv[tt][:],
                                xt_src(kt, tt),
                                wv_c[:, kt4, :],
                                start=(kt == 0), stop=(kt == KT - 1),
                            )
                for tt in range(4):
                    tok = vq * 4 + tt
                    for h in range(NH_LOC):
                        nc.vector.tensor_copy(
                            vhs[h][:, tok, :],
                            psv[tt][:, h * 128:(h + 1) * 128])
            with (
                tc.tile_pool(name="p1c", bufs=1) as p1c,
                tc.tile_pool(name="xtp", bufs=1) as xtp,
                tc.tile_pool(name="qkw", bufs=2) as qkw,
                tc.tile_pool(name="qkd", bufs=2) as qkd,
            ):
                perm_sb = p1c.tile([128, 128], F32R, tag="perm")
                fr_sb = p1c.tile([128, S], BF16, tag="fr")
                fis_sb = p1c.tile([128, S], BF16, tag="fis")

                xt_sb = xtp.tile([128, KT, S], BF16, tag="xt")
                xt_chunks = [(0, 4), (4, 12), (12, 22), (22, 32)]
                for ch, (k0, k1) in enumerate(xt_chunks):
                    nc.sync.dma_start(
                        xt_sb[:, k0:k1, :],
                        xT_d[k0 * 128:k1 * 128, :]
                        .rearrange("(kt p) s -> p kt s", p=128),
                    )
                    if ch == 0:
                        nc.scalar.dma_start(perm_sb[:], perm_d[:, :])
                        nc.scalar.dma_start(fr_sb[:], fr_d[:, :])
                        nc.scalar.dma_start(fis_sb[:], fis_d[:, :])
                        nc.scalar.dma_start(onesmat_sb[:], onesmat_d[:, :])
                        if causal:
                            nc.scalar.dma_start(maskt_sb[:], maskt_d[:, :])

                for oi in range(8):
                    w_src = wq_d if oi < 4 else wk_d
                    head = oi % 4
                    psums = [ps.tile([128, 512], F32, tag=f"a{j}",
                                     name=f"qkps{oi}_{j}", bufs=1)
                             for j in range(4)]
                    for wc in range(2):
                        w_c = qkw.tile([128, 16, 128], BF16, tag="w")
                        nc.scalar.dma_start(
                            w_c[:],
                            w_src[head, :, wc * 16:(wc + 1) * 16, :],
                        )
                        for kt8 in range(16):
                            kt = wc * 16 + kt8
                            for j in range(4):
                                nc.tensor.matmul(
                                    psums[j][:],
                                    w_c[:, kt8, :],
                                    xt_sb[:, kt, j * 512:(j + 1) * 512],
                                    start=(kt == 0), stop=(kt == KT - 1),
                                )
                    for j in range(4):
                        qt_sb = qkd.tile([128, 512], F32R, tag="qt")
                        nc.scalar.copy(qt_sb[:], psums[j][:])
                        swap_ps = ps.tile([128, 512], F32, tag="b",
                                          name=f"swap{oi}_{j}", bufs=2)
                        sw = nc.tensor.matmul(swap_ps[:], perm_sb[:], qt_sb[:])
                        if oi == 3 and j == 3:
                            anchor[0] = sw
                        tmp1 = qkd.tile([128, 512], F32, tag="t1")
                        nc.vector.tensor_mul(
                            tmp1[:], qt_sb[:],
                            fr_sb[:, j * 512:(j + 1) * 512])
                        tmp2 = qkd.tile([128, 512], F32, tag="t2")
                        nc.vector.tensor_mul(
                            tmp2[:], swap_ps[:],
                            fis_sb[:, j * 512:(j + 1) * 512])
                        if oi < 4:
                            rot = qkd.tile([128, 512], BF16, tag="rot")
                            nc.vector.tensor_add(rot[:], tmp1[:], tmp2[:])
                            nc.sync.dma_start(
                                qt_spill[head * 128:(head + 1) * 128,
                                         j * 512:(j + 1) * 512],
                                rot[:],
                            )
                        else:
                            nc.vector.tensor_add(
                                kts[head][:, j * 512:(j + 1) * 512],
                                tmp1[:], tmp2[:])

                # V quarters 0-1 directly off the resident x^T
                with tc.tile_pool(name="vw1", bufs=2) as vw1:
                    for vq01 in range(2):
                        emit_v_quarter(
                            vq01, vw1,
                            xt_src=(lambda kt, tt, _vq=vq01:
                                    xt_sb[:, kt,
                                          (_vq * 4 + tt) * 128:
                                          (_vq * 4 + tt + 1) * 128]))

            # ---------- Section 2: V projection interleaved with attention ----------
            with (
                tc.tile_pool(name="xtq", bufs=2) as xtqp,
                tc.tile_pool(name="aq", bufs=4) as aq,
                tc.tile_pool(name="apt", bufs=16) as apt,
                tc.tile_pool(name="awk", bufs=2) as awk,
                tc.tile_pool(name="wop", bufs=1) as wop,
                tc.tile_pool(name="woa", bufs=2) as woa,
                tc.tile_pool(name="woo", bufs=2) as woo,
                tc.tile_pool(name="vw", bufs=2) as vw,
            ):
                xtqp_h[0] = xtqp
                wo_sb = wop.tile([128, KT, DSH], BF16, tag="wo")
                for ch in range(4):
                    wdma = nc.sync.dma_start(
                        wo_sb[:, ch * 8:(ch + 1) * 8, :],
                        wo_d[:, ch * 8:(ch + 1) * 8, :],
                    )
                    tile.add_dep_helper(
                        wdma.ins, anchor[0].ins, sync=False,
                        reason="keep wo_sb load off the startup DMA burst")

                attn_last_pe = {}

                def emit_attn_head(qr, head):
                    kt_h = kts[head]
                    v_h = vhs[head]
                    q_sb = aq.tile([128, 512], BF16, tag="qsb",
                                   name=f"qsb{qr}_{head}")
                    nc.scalar.dma_start(
                        q_sb[:],
                        qt_spill[head * 128:(head + 1) * 128,
                                 qr * 512:(qr + 1) * 512])
                    nkt = (4 * qr + 4) if causal else QTILES
                    acc = awk.tile([128, 512], F32, tag="acc",
                                   name=f"acc{qr}_{head}", bufs=1)
                    pts = []
                    for kt in range(nkt):
                        ps_t = ps.tile([128, 512], F32, tag="b",
                                       name=f"st{qr}_{head}_{kt}", bufs=2)
                        nc.tensor.matmul(
                            ps_t[:], kt_h[:, kt * 128:(kt + 1) * 128],
                            q_sb[:])
                        pT = apt.tile([128, 512], BF16, tag="pT",
                                      name=f"pT{qr}_{head}_{kt}")
                        if general:
                            mt = awk.tile([128, 512], F32, tag="mt")
                            nc.sync.dma_start(
                                mt[:],
                                masktf_d[kt * 128:(kt + 1) * 128,
                                         qr * 512:(qr + 1) * 512])
                            msk = awk.tile([128, 512], F32, tag="msk")
                            nc.vector.scalar_tensor_tensor(
                                msk[:], ps_t[:], SCALE, mt[:],
                                op0=mybir.AluOpType.mult,
                                op1=mybir.AluOpType.add)
                            nc.scalar.activation(
                                pT[:], msk[:],
                                mybir.ActivationFunctionType.Exp)
                        elif not causal or kt < 4 * qr:
                            nc.scalar.activation(
                                pT[:], ps_t[:],
                                mybir.ActivationFunctionType.Exp,
                                scale=SCALE)
                        else:
                            for qtl in range(4):
                                qtile = qr * 4 + qtl
                                blk = slice(qtl * 128, (qtl + 1) * 128)
                                if qtile < kt:
                                    nc.vector.tensor_scalar_mul(
                                        pT[:, blk], ps_t[:, blk], 0.0)
                                elif qtile == kt:
                                    msk = awk.tile([128, 128], F32,
                                                   tag="mskd")
                                    nc.vector.scalar_tensor_tensor(
                                        msk[:], ps_t[:, blk], SCALE,
                                        maskt_sb[:],
                                        op0=mybir.AluOpType.mult,
                                        op1=mybir.AluOpType.add)
                                    nc.scalar.activation(
                                        pT[:, blk], msk[:],
                                        mybir.ActivationFunctionType.Exp)
                                else:
                                    nc.scalar.activation(
                                        pT[:, blk], ps_t[:, blk],
                                        mybir.ActivationFunctionType.Exp,
                                        scale=SCALE)
                        if kt == 0:
                            nc.vector.tensor_copy(acc[:], pT[:])
                        else:
                            nc.vector.tensor_add(acc[:], acc[:], pT[:])
                        pts.append(pT)

                    ps_pv = ps.tile([128, 512], F32, tag="c",
                                    name=f"pv{qr}_{head}", bufs=2)
                    for kt in range(nkt):
                        nc.tensor.matmul(
                            ps_pv[:], v_h[:, kt, :], pts[kt][:],
                            start=(kt == 0), stop=(kt == nkt - 1))
                    acc_r = awk.tile([128, 512], F32R, tag="accr",
                                     name=f"accr{qr}_{head}", bufs=1)
                    nc.vector.tensor_copy(acc_r[:], acc[:])
                    ps_rsb = ps.tile([128, 512], F32, tag="c",
                                     name=f"rsb{qr}_{head}", bufs=2)
                    rsb_mm = nc.tensor.matmul(ps_rsb[:], onesmat_sb[:],
                                              acc_r[:])
                    attn_last_pe[(qr, head)] = rsb_mm
                    rec_bc = awk.tile([128, 512], F32, tag="recb", bufs=1)
                    nc.vector.reciprocal(rec_bc[:], ps_rsb[:])
                    at_sb = awk.tile([128, 512], BF16, tag="at")
                    nc.vector.tensor_mul(at_sb[:], ps_pv[:], rec_bc[:])
                    nc.gpsimd.dma_start(
                        agi[qr][head * 128:(head + 1) * 128, :], at_sb[:])

                def emit_ag(qr):
                    nc.gpsimd.collective_compute(
                        "AllGather",
                        mybir.AluOpType.bypass,
                        replica_groups=[list(range(N_CORES))],
                        ins=[agi[qr][:].opt()],
                        outs=[ago[qr][:].opt()],
                    )

                # head-pair sliced AllGather: hp=0 gathers local heads 0-1,
                # hp=1 heads 2-3. Output rows: core-major [8 x 256].
                agoh = [dram.tile([2048, 512], BF16, addr_space="Shared",
                                  name=f"agoh{hp}") for hp in range(2)]

                def emit_ag_hp(qr, hp):
                    nc.gpsimd.collective_compute(
                        "AllGather",
                        mybir.AluOpType.bypass,
                        replica_groups=[list(range(N_CORES))],
                        ins=[agi[qr][hp * 256:(hp + 1) * 256, :].opt()],
                        outs=[agoh[hp][:].opt()],
                    )

                wo_hp_ps = {}

                def emit_wo_hp(r, hp, after=None):
                    # accumulate the 16 d-tiles carried by head-pair hp
                    after_inst = attn_last_pe.get(after)
                    if hp == 0:
                        wo_hp_ps[r] = [ps.tile([128, 512], F32, tag=f"a{q}",
                                               name=f"wops{r}h_{q}", bufs=1)
                                       for q in range(4)]
                    ps_os3 = wo_hp_ps[r]
                    first_mm = [True]
                    for cc in range(N_CORES):
                        atqf = woa.tile([128, 2, 512], BF16, tag="atqh",
                                        name=f"atqh{r}_{hp}_{cc}", bufs=2)
                        nc.sync.dma_start(
                            atqf[:],
                            agoh[hp][cc * 256:(cc + 1) * 256, :]
                            .rearrange("(dt p) q -> p dt q", p=128),
                        )
                        for qtl in range(4):
                            for dt in range(2):
                                gdt = cc * 4 + hp * 2 + dt
                                mm = nc.tensor.matmul(
                                    ps_os3[qtl][:],
                                    atqf[:, dt, qtl * 128:(qtl + 1) * 128],
                                    wo_sb[:, gdt, :],
                                    start=(hp == 0 and cc == 0 and dt == 0),
                                    stop=(hp == 1 and cc == N_CORES - 1
                                          and dt == 1))
                                if first_mm[0] and after_inst is not None:
                                    tile.add_dep_helper(
                                        mm.ins, after_inst.ins, sync=False,
                                        reason="order wo_hp after attn")
                                    first_mm[0] = False
                    if hp == 1:
                        ps_os3 = wo_hp_ps[r]
                        for qtl in range(4):
                            qt = r * 4 + qtl
                            o_sb = woo.tile([128, 512], F32, tag="osb",
                                            name=f"osb{qt}")
                            nc.vector.tensor_copy(o_sb[:], ps_os3[qtl][:])
                            nc.sync.dma_start(
                                out_d[qt * 128:(qt + 1) * 128, :], o_sb[:])

                def emit_wo(r, after=None):
                    after_inst = attn_last_pe.get(after)
                    ps_os = [ps.tile([128, 512], F32, tag=f"a{qtl}",
                                     name=f"wops{r}_{qtl}", bufs=1)
                             for qtl in range(4)]
                    first_mm = [True]
                    for hh in range(4):
                        atqf = woa.tile([128, 8, 512], BF16, tag="atqf",
                                        name=f"atqf{r}_{hh}")
                        nc.scalar.dma_start(
                            atqf[:],
                            ago[r][hh * 1024:(hh + 1) * 1024, :]
                            .rearrange("(dt p) q -> p dt q", p=128),
                        )
                        for qtl in range(4):
                            for dt in range(8):
                                gdt = hh * 8 + dt
                                mm = nc.tensor.matmul(
                                    ps_os[qtl][:],
                                    atqf[:, dt, qtl * 128:(qtl + 1) * 128],
                                    wo_sb[:, gdt, :],
                                    start=(gdt == 0), stop=(gdt == KT - 1))
                                if first_mm[0] and after_inst is not None:
                                    tile.add_dep_helper(
                                        mm.ins, after_inst.ins, sync=False,
                                        reason="order wo after next attn qr")
                                    first_mm[0] = False
                    for qtl in range(4):
                        qt = r * 4 + qtl
                        o_sb = woo.tile([128, 512], F32, tag="osb",
                                        name=f"osb{qt}")
                        nc.vector.tensor_copy(o_sb[:], ps_os[qtl][:])
                        nc.sync.dma_start(
                            out_d[qt * 128:(qt + 1) * 128, :], o_sb[:])

                # ---------- emission schedule ----------
                if causal and do_attn:
                    emit_attn_head(0, 0)
                    emit_attn_head(0, 1)
                    emit_v_quarter(2, vw)
                    emit_attn_head(0, 2)
                    emit_attn_head(0, 3)
                    if do_ag:
                        emit_ag(0)
                    emit_attn_head(1, 0)
                    emit_attn_head(1, 1)
                    emit_v_quarter(3, vw)
                    emit_attn_head(1, 2)
                    emit_attn_head(1, 3)
                    if do_ag:
                        emit_ag(1)
                    emit_attn_head(2, 0)
                    emit_attn_head(2, 1)
                    if do_ag and do_wo:
                        emit_wo(0, after=(2, 1))
                    emit_attn_head(2, 2)
                    emit_attn_head(2, 3)
                    if do_ag:
                        emit_ag(2)
                    emit_attn_head(3, 0)
                    emit_attn_head(3, 1)
                    if do_ag and do_wo:
                        emit_wo(1, after=(3, 1))
                    emit_attn_head(3, 2)
                    emit_attn_head(3, 3)
                    if do_ag:
                        emit_ag(3)
                        if do_wo:
                            emit_wo(2, after=(3, 3))
                            emit_wo(3)
                else:
                    for vq in range(2, 4):
                        emit_v_quarter(vq, vw)
                    if do_attn:
                        for qr in range(QRANGES):
                            for head in range(NH_LOC):
                                emit_attn_head(qr, head)
                            if do_ag:
                                emit_ag(qr)
                                if do_wo and qr > 0:
                                    emit_wo(qr - 1)
                        if do_wo:
                            emit_wo(QRANGES - 1)

                if debug_stop == "qkv":
                    for h in range(NH_LOC):
                        nc.sync.dma_start(
                            dbg_d[DSH + h * 128:DSH + (h + 1) * 128, :],
                            kts[h][:])
                        nc.gpsimd.dma_start(
                            out_d[:, h * 128:(h + 1) * 128]
                            .rearrange("(tt p) d -> p tt d", p=128),
                            vhs[h][:])
                    nc.sync.dma_start(dbg_d[0:DSH, :], qt_spill[:, :])
                if debug_stop == "attn":
                    for r in range(4):
                        nc.sync.dma_start(
                            dbg_d[:, r * 512:(r + 1) * 512], agi[r][:, :])
                if debug_stop == "ag":
                    for r in range(4):
                        nc.sync.dma_start(
                            dbg_d[:, r * 512:(r + 1) * 512],
                            ago[r][0:DSH, :])

    nc.compile()
    return nc


def _get_program(mode, debug_stop=None):
    key = (mode, debug_stop)
    if key not in _PROGRAMS:
        _PROGRAMS[key] = _build_program(mode, debug_stop)
    return _PROGRAMS[key]


def _prep_inputs(x, wq, wk, wv, wo, freqs_real, freqs_imag, mask):
    """Host-side shard/layout prep. Returns (mode, in_maps)."""
    x = np.asarray(x, dtype=np.float32)
    wq = np.asarray(wq, dtype=np.float32)
    wk = np.asarray(wk, dtype=np.float32)
    wv = np.asarray(wv, dtype=np.float32)
    wo = np.asarray(wo, dtype=np.float32)
    fr = np.asarray(freqs_real, dtype=np.float32)
    fi = np.asarray(freqs_imag, dtype=np.float32)
    m = np.asarray(mask, dtype=np.float32).reshape(S, S)

    causal_ref = np.triu(np.full((S, S), np.float32(-1e9), dtype=np.float32), k=1)
    if np.array_equal(m, causal_ref):
        mode = "causal"
    elif not m.any():
        mode = "nomask"
    else:
        mode = "general"

    xT = np.ascontiguousarray(x.reshape(S, D).T)  # [D, S]
    xT_bf = xT.astype(ml_dtypes.bfloat16)

    # evens-first permutation of each head's 128 dims (for RoPE pair layout)
    idx = np.concatenate([np.arange(0, HD, 2), np.arange(1, HD, 2)])
    cols = np.concatenate([h * HD + idx for h in range(32)])
    wq_p = wq[:, cols]
    wk_p = wk[:, cols]

    fr128 = np.ascontiguousarray(np.concatenate([fr.T, fr.T], axis=0))   # [128, S]
    fis128 = np.ascontiguousarray(np.concatenate([-fi.T, fi.T], axis=0))

    perm = np.zeros((128, 128), dtype=np.float32)
    perm[np.arange(128), (np.arange(128) + 64) % 128] = 1.0

    onesmat = np.ones((128, 128), dtype=np.float32)

    in_maps = []
    for c in range(N_CORES):
        sl = slice(c * DSH, (c + 1) * DSH)
        def _wtile(a):
            # [D, C] -> [128p, KT, C] matching the SBUF tile layout
            return np.ascontiguousarray(
                a.reshape(KT, 128, a.shape[1]).transpose(1, 0, 2)
            ).astype(ml_dtypes.bfloat16)

        def _whead(a):
            # [D, 512] -> [NH_LOC, 128p, KT, HD]
            return np.ascontiguousarray(np.stack([
                _wtile(a[:, h * HD:(h + 1) * HD]) for h in range(NH_LOC)
            ]))

        im = {
            "xT": xT_bf,
            "wq": _whead(wq_p[:, sl]),
            "wk": _whead(wk_p[:, sl]),
            "wv": _wtile(wv[:, sl]),
            "wo": _wtile(wo[:, sl]),
            "fr128": fr128.astype(ml_dtypes.bfloat16),
            "fis128": fis128.astype(ml_dtypes.bfloat16),
            "perm": perm,
            "onesmat": onesmat,
        }
        if mode == "causal":
            # mask tile in [k, q] layout: valid iff k <= q
            maskt = np.where(
                np.arange(128)[:, None] <= np.arange(128)[None, :],
                np.float32(0.0), np.float32(-1e9)).astype(np.float32)
            im["maskt"] = maskt
        if mode == "general":
            im["masktf"] = np.ascontiguousarray(m.T)
        in_maps.append(im)
    return mode, in_maps


def kernel(x, wq, wk, wv, wo, cache_k, cache_v, freqs_real, freqs_imag,
           mask, start_pos, **_unused):
    assert int(start_pos) == 0, "kernel hardcodes start_pos=0"
    mode, in_maps = _prep_inputs(x, wq, wk, wv, wo, freqs_real, freqs_imag, mask)
    nc = _get_program(mode)
    res = run_bass_kernel_spmd(nc, in_maps, core_ids=list(range(N_CORES)))
    out = np.concatenate([res.results[c]["out"] for c in range(N_CORES)], axis=1)
    return out.reshape(1, S, D).astype(np.float32)
